# revision 15
# baseline (speedup 1.0000x reference)
"""Trainium2 Bass kernel for the combined loss (KL + CE + InfoNCE + focal + adv CE).

Strategy (8 NeuronCores, data-parallel over the batch):
  - The three [4096, 1000] tensors (output / master_net_pred / output_adv) and the
    targets are sharded by rows: 512 rows per core.  They are cast to bf16 on the
    host to halve HBM traffic; all on-device accumulation stays fp32.
  - InfoNCE: feats = concat(feat_pooled, feat_pooled_masked) -> [8192, 256],
    bf16, transposed to [256, 8192] with columns rolled per-core so the core's
    own 1024 query rows sit at fixed column positions {0..512, 4096..4608}.
    Each core normalizes the full matrix on-device (bf16 squares on DVE ->
    bf16 ones-matmul colsum -> Ln -> Exp(-0.5 ln) -> bf16 scale on DVE),
    computes its 1024x8192 stripe of the cosine-similarity Gram matrix on the
    PE in bf16 (1024-wide moving operand), and streams a no-max bounded-logit
    logsumexp via ScalarE fused exp+row-accumulate.  Diagonal masked by a
    -1e9*I matmul accumulated into the PSUM group; positives picked off the
    exp tile with an identity-mask scalar_tensor_tensor.
  - The CE/KL/focal per-row stats are interleaved (in program order) with the
    normalization phase so the ScalarE never idles during the feature DMA /
    PE ramp-up.
  - Each core emits 5 partial sums; the host sums the 8x[8] outputs and applies
    the loss weights.
"""

import numpy as np

import concourse.bacc as bacc
import concourse.tile as tile
from concourse import mybir
from concourse.bass_utils import run_bass_kernel_spmd

F32 = mybir.dt.float32
I32 = mybir.dt.int32
BF16 = mybir.dt.bfloat16
AF = mybir.ActivationFunctionType
ALU = mybir.AluOpType
AX = mybir.AxisListType

NCORES = 8
B, C, D = 4096, 1000, 256
RB = B // NCORES          # 512 rows of the [B, C] tensors per core
NT = RB // 128            # 4 row-tiles per core
N2 = 2 * B                # 8192 infoNCE rows
CH = 2048                 # column chunk for the Gram stripe
NCH = N2 // CH            # 4 chunks
SUB = 512                 # matmul moving free dim (PSUM bank limit)
NSUB = CH // SUB          # 4
# q-block column starts after the per-core column roll (identical on all cores)
QCOLS = [0, 128, 256, 384, N2 // 2, N2 // 2 + 128, N2 // 2 + 256, N2 // 2 + 384]
QB = len(QCOLS)

KL_TEMP = 4.0
KL_INTERP = 0.5
NCE_TEMP = 0.07
# Diagonal mask: -3.0 pushes the self-logit to (s-3)/T <= -28.6 -> exp ~ 4e-13,
# negligible in every row sum, while keeping the Schraudolph integer positive.
NEG_BIG = -3.0

# Schraudolph fast-exp (DVE): exp(s/T) ~ bitcast_f32(round(EXP_A*s + EXP_B)).
# EXP_B tuned for zero-mean relative error (max ~3.9%) over uniform logits.
EXP_A = float(2.0 ** 23 / (NCE_TEMP * float(np.log(2.0))))
EXP_B = float(127.0 * 2 ** 23 - 484088.0)
# (qi, chx) Gram chunks whose exp+rowsum run on the DVE instead of ScalarE.
DVE_CHUNKS = {(qi, chx) for qi in range(4) for chx in (1, 3)} | \
             {(qi, 1) for qi in (4, 5, 6)}


class _PinnedBacc(bacc.Bacc):
    """Bacc whose activation-table chooser only sees the one table set that
    serves every function this kernel uses (ln/exp/copy) — avoids the
    ~1.5us ACT_TABLE_LOAD thrash between the `exp_and_others` and
    `natural_log` sets that the default chooser picks."""

    _PIN = "natural_log_exp_and_others"

    def insert_act_table_loads(self):
        from concourse.hw_specs import get_activation_tables
        from concourse.bass import _bass_rust
        import concourse.mybir as mb

        has_activation = any(
            isinstance(i, mb.InstActivation)
            for b in self.main_func.blocks
            for i in b.instructions
        )
        if not has_activation:
            return
        tables = list(get_activation_tables(self.m.arch).items())
        pinned = [
            (name, funcs if name == self._PIN else set())
            for name, funcs in tables
        ]
        _bass_rust.insert_act_table_loads(self, pinned)


def _build_module():
    nc = _PinnedBacc("TRN2", target_bir_lowering=False, debug=False)

    o_d = nc.dram_tensor("o", [RB, C], BF16, kind="ExternalInput")
    m_d = nc.dram_tensor("m", [RB, C], BF16, kind="ExternalInput")
    a_d = nc.dram_tensor("a", [RB, C], BF16, kind="ExternalInput")
    tg_d = nc.dram_tensor("tg", [128, NT], F32, kind="ExternalInput")
    ta_d = nc.dram_tensor("ta", [128, NT], F32, kind="ExternalInput")
    ft_d = nc.dram_tensor("ft", [256, N2], BF16, kind="ExternalInput")
    res_d = nc.dram_tensor("res", [8, 1], F32, kind="ExternalOutput")

    import ml_dtypes
    iota_np = np.tile(np.arange(C, dtype=np.float32), (128, 1))
    ident_np = np.eye(128, dtype=np.float32)
    identb_np = np.eye(128).astype(ml_dtypes.bfloat16)
    negidb_np = (NEG_BIG * np.eye(128)).astype(ml_dtypes.bfloat16)
    onesb_np = np.ones((128, 128)).astype(ml_dtypes.bfloat16)
    ones_np = np.ones((128, 1), dtype=np.float32)
    iota_d = nc.inline_tensor(iota_np, "iota_c")
    ident_d = nc.inline_tensor(ident_np, "ident_c")
    identb_d = nc.inline_tensor(identb_np, "identb_c")
    negidb_d = nc.inline_tensor(negidb_np, "negidb_c")
    onesb_d = nc.inline_tensor(onesb_np, "onesb_c")
    ones_d = nc.inline_tensor(ones_np, "ones_c")

    with tile.TileContext(nc) as tc:
        with (
            tc.tile_pool(name="persist", bufs=1) as persist,
            tc.tile_pool(name="scr", bufs=4) as scrp,
            tc.tile_pool(name="norm", bufs=2) as normp,
            tc.tile_pool(name="es", bufs=3) as esp,
            tc.tile_pool(name="vec", bufs=1) as vecp,
            tc.tile_pool(name="ps", bufs=2, space="PSUM") as psp,
        ):
            dma = nc.default_dma_engine.dma_start

            # ---- constants + input DMAs (all issued up front) ----
            iota_t = persist.tile([128, C], F32, tag="iota")
            dma(out=iota_t[:], in_=iota_d[:])
            ident_t = persist.tile([128, 128], F32, tag="ident")
            dma(out=ident_t[:], in_=ident_d[:])
            identb_t = persist.tile([128, 128], BF16, tag="identb")
            dma(out=identb_t[:], in_=identb_d[:])
            negidb_t = persist.tile([128, 128], BF16, tag="negidb")
            dma(out=negidb_t[:], in_=negidb_d[:])
            onesb_t = persist.tile([128, 128], BF16, tag="onesb")
            dma(out=onesb_t[:], in_=onesb_d[:])
            ones_t = persist.tile([128, 1], F32, tag="ones")
            dma(out=ones_t[:], in_=ones_d[:])
            tg_t = persist.tile([128, NT], F32, tag="tg")
            dma(out=tg_t[:], in_=tg_d[:])
            ta_t = persist.tile([128, NT], F32, tag="ta")
            dma(out=ta_t[:], in_=ta_d[:])

            o_ts, m_ts, a_ts = [], [], []
            for t in range(NT):
                rsl = slice(t * 128, (t + 1) * 128)
                o_t = persist.tile([128, C], BF16, tag=f"o{t}")
                dma(out=o_t[:], in_=o_d[rsl, :])
                m_t = persist.tile([128, C], BF16, tag=f"m{t}")
                dma(out=m_t[:], in_=m_d[rsl, :])
                a_t = persist.tile([128, C], BF16, tag=f"a{t}")
                dma(out=a_t[:], in_=a_d[rsl, :])
                o_ts.append(o_t)
                m_ts.append(m_t)
                a_ts.append(a_t)

            h0 = persist.tile([128, N2], BF16, tag="h0")
            h1 = persist.tile([128, N2], BF16, tag="h1")
            for j in range(NCH):
                cs = slice(j * CH, (j + 1) * CH)
                dma(out=h0[:, cs], in_=ft_d[0:128, cs])
                dma(out=h1[:, cs], in_=ft_d[128:256, cs])

            # ---- per-row CE / KL / focal / adv stat vectors ----
            # ST4 columns: [S1 | SA | ST | SM] so that ln(S1|SA) is one
            # contiguous [128, 2*NT] slice matched against G2 = [GO | GA].
            ST4 = vecp.tile([128, 4 * NT], F32, tag="ST4")
            S1 = ST4[:, 0 * NT:1 * NT]                 # sum exp(o)
            SA = ST4[:, 1 * NT:2 * NT]                 # sum exp(a)
            ST = ST4[:, 2 * NT:3 * NT]                 # sum exp(o/T)
            SM = ST4[:, 3 * NT:4 * NT]                 # sum exp(m/T)
            PP = vecp.tile([128, NT], F32, tag="PP")   # sum exp(m/T)*(m-o)
            G2 = vecp.tile([128, 2 * NT], F32, tag="G2")
            GO = G2[:, 0:NT]                           # o[target]
            GA = G2[:, NT:2 * NT]                      # a[target_adv]

            def cekl_tile(t):
                o_t, m_t, a_t = o_ts[t], m_ts[t], a_ts[t]
                e1 = scrp.tile([128, C], F32, tag="scr1000")
                nc.scalar.activation(e1[:], o_t[:], AF.Exp, scale=1.0,
                                     accum_out=S1[:, t:t + 1])
                e2 = scrp.tile([128, C], F32, tag="scr1000")
                nc.scalar.activation(e2[:], o_t[:], AF.Exp,
                                     scale=float(1.0 / KL_TEMP),
                                     accum_out=ST[:, t:t + 1])
                em_t = scrp.tile([128, C], F32, tag="scr1000")
                nc.scalar.activation(em_t[:], m_t[:], AF.Exp,
                                     scale=float(1.0 / KL_TEMP),
                                     accum_out=SM[:, t:t + 1])
                e3 = scrp.tile([128, C], F32, tag="scr1000")
                nc.scalar.activation(e3[:], a_t[:], AF.Exp, scale=1.0,
                                     accum_out=SA[:, t:t + 1])

                d_t = scrp.tile([128, C], F32, tag="scr1000")
                nc.vector.tensor_sub(d_t[:], m_t[:], o_t[:])
                pr = scrp.tile([128, C], F32, tag="scr1000")
                nc.vector.scalar_tensor_tensor(
                    out=pr[:], in0=d_t[:], scalar=1.0, in1=em_t[:],
                    op0=ALU.mult, op1=ALU.mult, accum_out=PP[:, t:t + 1])
                g1 = scrp.tile([128, C], F32, tag="scr1000")
                nc.vector.scalar_tensor_tensor(
                    out=g1[:], in0=iota_t[:], scalar=tg_t[:, t:t + 1],
                    in1=o_t[:], op0=ALU.is_equal, op1=ALU.mult,
                    accum_out=GO[:, t:t + 1])
                g2 = scrp.tile([128, C], F32, tag="scr1000")
                nc.vector.scalar_tensor_tensor(
                    out=g2[:], in0=iota_t[:], scalar=ta_t[:, t:t + 1],
                    in1=a_t[:], op0=ALU.is_equal, op1=ALU.mult,
                    accum_out=GA[:, t:t + 1])

            # ---- normalization, chunk-pipelined, interleaved with CE/KL ----
            h0b = persist.tile([128, N2], BF16, tag="h0b")
            h1b = persist.tile([128, N2], BF16, tag="h1b")
            for j in range(NCH):
                cs = slice(j * CH, (j + 1) * CH)
                cekl_tile(j)
                s0 = normp.tile([128, CH], BF16, tag="sq")
                nc.vector.tensor_mul(s0[:], h0[:, cs], h0[:, cs])
                s1 = normp.tile([128, CH], BF16, tag="sq")
                nc.vector.tensor_mul(s1[:], h1[:, cs], h1[:, cs])
                ps_n = psp.tile([128, CH], F32, tag="ps")
                for s in range(NSUB):
                    sl = slice(s * SUB, (s + 1) * SUB)
                    nc.tensor.matmul(ps_n[:, sl], onesb_t[:], s0[:, sl],
                                     start=True, stop=False)
                    nc.tensor.matmul(ps_n[:, sl], onesb_t[:], s1[:, sl],
                                     start=False, stop=True)
                lns = normp.tile([128, CH], F32, tag="lns")
                nc.scalar.activation(lns[:], ps_n[:], AF.Ln)
                rnj = normp.tile([128, CH], BF16, tag="rnj")
                nc.scalar.activation(rnj[:], lns[:], AF.Exp, scale=-0.5)
                nc.vector.tensor_mul(h0b[:, cs], h0[:, cs], rnj[:])
                nc.vector.tensor_mul(h1b[:, cs], h1[:, cs], rnj[:])

            # ---- InfoNCE stripe: 8 q-blocks x full 8192 columns ----
            NP2 = vecp.tile([128, 2 * QB], F32, tag="NP2")
            rs_all = NP2[:, 0:QB]                      # sum exp(logits)
            posx_all = NP2[:, QB:2 * QB]               # exp(pos logit)
            for qi, q0 in enumerate(QCOLS):
                p0 = (q0 + N2 // 2) % N2
                lhsT0 = h0b[:, q0:q0 + 128]
                lhsT1 = h1b[:, q0:q0 + 128]
                rsp = scrp.tile([128, NCH], F32, tag="rsp")
                for chx in range(NCH):
                    base = chx * CH
                    ps_t = psp.tile([128, CH], F32, tag="ps")
                    for s in range(NSUB):
                        c0 = base + s * SUB
                        sl = slice(s * SUB, (s + 1) * SUB)
                        nc.tensor.matmul(ps_t[:, sl], lhsT0, h0b[:, c0:c0 + SUB],
                                         start=True, stop=False)
                        if c0 <= q0 < c0 + SUB:
                            off = s * SUB + (q0 - c0)
                            nc.tensor.matmul(ps_t[:, off:off + 128], negidb_t[:],
                                             identb_t[:], start=False, stop=False,
                                             skip_group_check=True)
                        nc.tensor.matmul(ps_t[:, sl], lhsT1, h1b[:, c0:c0 + SUB],
                                         start=False, stop=True)
                    if (qi, chx) in DVE_CHUNKS:
                        # Schraudolph fast-exp on the DVE: one convert-on-write
                        # tensor_scalar + one reduce over the bitcast view.
                        ei = esp.tile([128, CH], I32, tag="esi")
                        nc.vector.tensor_scalar(ei[:], ps_t[:], EXP_A, EXP_B,
                                                op0=ALU.mult, op1=ALU.add)
                        nc.vector.reduce_sum(rsp[:, chx:chx + 1],
                                             ei[:].bitcast(F32), axis=AX.X)
                        continue
                    es = esp.tile([128, CH], BF16, tag="es")
                    nc.scalar.activation(es[:], ps_t[:], AF.Exp,
                                         scale=float(1.0 / NCE_TEMP),
                                         accum_out=rsp[:, chx:chx + 1])
                    if base <= p0 < base + CH:
                        # pick exp(pos) off the exp tile via the identity mask
                        poff = p0 - base
                        pscr = scrp.tile([128, 128], F32, tag="posscr")
                        nc.vector.scalar_tensor_tensor(
                            out=pscr[:], in0=es[:, poff:poff + 128],
                            scalar=1.0, in1=ident_t[:],
                            op0=ALU.mult, op1=ALU.mult,
                            accum_out=posx_all[:, qi:qi + 1])
                nc.vector.reduce_sum(rs_all[:, qi:qi + 1], rsp[:], axis=AX.X)

            # ---- epilogue on [128, NT] / [128, QB] stat vectors ----
            lse4 = vecp.tile([128, 4 * NT], F32, tag="lse4")
            nc.scalar.activation(lse4[:], ST4[:], AF.Ln)  # [ln S1|SA|ST|SM]
            lse1 = lse4[:, 0 * NT:1 * NT]
            lsea = lse4[:, 1 * NT:2 * NT]
            lseT = lse4[:, 2 * NT:3 * NT]
            lsem = lse4[:, 3 * NT:4 * NT]

            cad = vecp.tile([128, 2 * NT], F32, tag="cad")  # [ce | adv]
            nc.vector.tensor_sub(cad[:], lse4[:, 0:2 * NT], G2[:])
            ce = cad[:, 0:NT]
            adv = cad[:, NT:2 * NT]

            # kl_row = PP/(T*SM) - lsem + lseT
            invSM = vecp.tile([128, NT], F32, tag="invSM")
            nc.vector.reciprocal(invSM[:], SM[:])
            kl = vecp.tile([128, NT], F32, tag="kl")
            nc.vector.tensor_mul(kl[:], PP[:], invSM[:])
            nc.vector.tensor_scalar_mul(kl[:], kl[:], float(1.0 / KL_TEMP))
            nc.vector.tensor_sub(kl[:], kl[:], lsem[:])
            nc.vector.tensor_add(kl[:], kl[:], lseT[:])

            # focal_row = (1-pt)^gamma * ce,  pt = exp(-ce)
            pt = vecp.tile([128, NT], F32, tag="pt")
            nc.scalar.activation(pt[:], ce[:], AF.Exp, scale=-1.0)
            c1 = vecp.tile([128, NT], F32, tag="c1")
            nc.vector.tensor_scalar(c1[:], pt[:], 0.5, None, op0=ALU.is_lt)
            c2 = vecp.tile([128, NT], F32, tag="c2")
            nc.vector.tensor_scalar(c2[:], pt[:], 0.2, None, op0=ALU.is_lt)
            gam = vecp.tile([128, NT], F32, tag="gam")
            nc.vector.tensor_add(gam[:], c1[:], c2[:])
            nc.vector.tensor_scalar(gam[:], gam[:], 2.0, 1.0,
                                    op0=ALU.mult, op1=ALU.add)
            u = vecp.tile([128, NT], F32, tag="u")       # 1 - pt
            nc.vector.tensor_scalar(u[:], pt[:], -1.0, 1.0,
                                    op0=ALU.mult, op1=ALU.add)
            lg = vecp.tile([128, NT], F32, tag="lg")
            nc.scalar.activation(lg[:], u[:], AF.Ln)
            w = vecp.tile([128, NT], F32, tag="w")
            nc.vector.tensor_mul(w[:], gam[:], lg[:])
            nc.scalar.activation(w[:], w[:], AF.Exp)     # (1-pt)^gamma
            foc = vecp.tile([128, NT], F32, tag="foc")
            nc.vector.tensor_mul(foc[:], w[:], ce[:])

            # nce_row = ln(rowsum) - ln(exp(pos)); one Ln over [rs|posx]
            lnp = vecp.tile([128, 2 * QB], F32, tag="lnp")
            nc.scalar.activation(lnp[:], NP2[:], AF.Ln)
            nce = vecp.tile([128, QB], F32, tag="nce")
            nc.vector.tensor_sub(nce[:], lnp[:, 0:QB], lnp[:, QB:2 * QB])

            # ---- reduce to 5 partial sums, then across partitions via PE ----
            acc = vecp.tile([128, 8], F32, tag="acc")
            nc.vector.reduce_sum(acc[:, 0:1], kl[:], axis=AX.X)
            nc.vector.reduce_sum(acc[:, 1:2], ce[:], axis=AX.X)
            nc.vector.reduce_sum(acc[:, 2:3], adv[:], axis=AX.X)
            nc.vector.reduce_sum(acc[:, 3:4], foc[:], axis=AX.X)
            nc.vector.reduce_sum(acc[:, 4:5], nce[:], axis=AX.X)
            nc.vector.memset(acc[:, 5:8], 0.0)

            ps_f = psp.tile([8, 1], F32, tag="ps")
            nc.tensor.matmul(ps_f[:], acc[:], ones_t[:],
                             start=True, stop=True)
            out_sb = vecp.tile([8, 1], F32, tag="out_sb")
            nc.scalar.copy(out_sb[:], ps_f[:])
            dma(out=res_d[:], in_=out_sb[:])

    nc.compile()
    return nc


_NC = None


def _get_nc():
    global _NC
    if _NC is None:
        _NC = _build_module()
    return _NC


def _prep_inputs(output, target, master_net_pred, feat_pooled,
                 feat_pooled_masked, output_adv, target_adv):
    import ml_dtypes
    BF = ml_dtypes.bfloat16
    o = np.ascontiguousarray(np.asarray(output, dtype=np.float32).astype(BF))
    m = np.ascontiguousarray(
        np.asarray(master_net_pred, dtype=np.float32).astype(BF))
    a = np.ascontiguousarray(np.asarray(output_adv, dtype=np.float32).astype(BF))
    tg = np.asarray(target).astype(np.int64)
    ta = np.asarray(target_adv).astype(np.int64)
    f0 = np.asarray(feat_pooled, dtype=np.float32)
    f1 = np.asarray(feat_pooled_masked, dtype=np.float32)
    feats = np.concatenate([f0, f1], axis=0).astype(BF)  # [2B, D]

    in_maps = []
    for cc in range(NCORES):
        sl = slice(cc * RB, (cc + 1) * RB)
        roll = np.concatenate([np.arange(cc * RB, B), np.arange(0, cc * RB)])
        order = np.concatenate([roll, B + roll])
        ftc = np.ascontiguousarray(feats[order].T)  # [D, 2B], cols rolled
        in_maps.append({
            "o": o[sl].copy(),
            "m": m[sl].copy(),
            "a": a[sl].copy(),
            "tg": np.ascontiguousarray(
                tg[sl].reshape(NT, 128).T.astype(np.float32)),
            "ta": np.ascontiguousarray(
                ta[sl].reshape(NT, 128).T.astype(np.float32)),
            "ft": ftc,
        })
    return in_maps


def _combine(results):
    r = np.zeros(8, dtype=np.float64)
    for rr in results:
        r += rr["res"].reshape(-1).astype(np.float64)
    kl_mean = r[0] / (B * C)
    ce_mean = r[1] / B
    adv_mean = r[2] / B
    foc_mean = r[3] / B
    nce_mean = r[4] / (2 * B)
    loss = (KL_INTERP * KL_TEMP * KL_TEMP) * kl_mean \
        + (1.0 - KL_INTERP) * ce_mean + nce_mean + foc_mean + adv_mean
    return np.asarray([loss], dtype=np.float32)


def kernel(**inputs):
    in_maps = _prep_inputs(**inputs)
    out = run_bass_kernel_spmd(_get_nc(), in_maps,
                               core_ids=list(range(NCORES)))
    return _combine(out.results)


if __name__ == "__main__":
    rng = np.random.default_rng(0)
    ins = {
        "output": rng.standard_normal((B, C), dtype=np.float32),
        "target": rng.integers(0, C, size=(B,)),
        "master_net_pred": rng.standard_normal((B, C), dtype=np.float32),
        "feat_pooled": rng.standard_normal((B, D), dtype=np.float32),
        "feat_pooled_masked": rng.standard_normal((B, D), dtype=np.float32),
        "output_adv": rng.standard_normal((B, C), dtype=np.float32),
        "target_adv": rng.integers(0, C, size=(B,)),
    }
    print(kernel(**ins))
